# revision 6
# baseline (speedup 1.0000x reference)
"""CondensedLinearFineGrained on 8 TRN2 NeuronCores.

Math: out[b,o] = sum_k W[o,k] * input[b, mask[o,k]] + bias[o]
with B=256, IN_F=4096, OUT_F=4096, K=256.

Strategy
--------
Reformulate as a dense matmul:  out = input @ A^T + bias  where
A[o,f] = sum_{k: mask[o,k]==f} W[o,k]  (duplicates within a row are summed).

Materializing the gathered tensor [B, OUT_F, K] would push 268M elements
through the vector/gpsimd engines (hundreds of us); the dense reformulation
does 16x the MACs but on the 128x128 systolic array (~13.7us/core bf16),
and A is only 16x the size of W.

Sharding: output neurons, 512 per core. Per core:
  - input^T arrives as bf16 f-major tiles [128f x 32t x 256b] (2MB)
  - A^T f-tiles [128f x 512o] bf16 are built on-device by gpsimd
    local_scatter from host-repacked CSC (per-feature (o,weight) lists,
    deduped, -1-padded, int16 indices), EXCEPT the first N_DENSE_TILES
    f-tiles which are shipped pre-densified from the host to balance the
    Pool-engine scatter time against spare DMA bandwidth.
  - TensorE accumulates psum[128b x 512o] over the 32 f-tiles; PSUM is
    seeded with bias via a K=1 f32r matmul (ones^T @ bias broadcasts it
    across partitions).
"""

import numpy as np
import ml_dtypes

B = 256
IN_F = 4096
OUT_F = 4096
K = 256
N_CORES = 8
O_SH = OUT_F // N_CORES  # 512 output rows per core
NT = IN_F // 128         # 32 feature tiles
NB = B // 128            # 2 batch tiles

# f-tiles [0, N_DENSE_TILES) are DMA'd pre-densified; the rest are scattered
# on-device by gpsimd. Tuned so Pool time ~ DMA time.
N_DENSE_TILES = 14

_BF16 = ml_dtypes.bfloat16

_prog_cache = {}


def _build_program(wpad: int, n_dense: int):
    """Build + compile the SPMD Bass program. Cached on (wpad, n_dense)."""
    key = (wpad, n_dense)
    if key in _prog_cache:
        return _prog_cache[key]

    from concourse import bacc, tile, mybir

    nt_s = NT - n_dense  # number of scattered f-tiles

    nc = bacc.Bacc("TRN2", target_bir_lowering=False, debug=False)
    dt = mybir.dt

    inT_d = nc.dram_tensor("inT", [128, NT, B], dt.bfloat16, kind="ExternalInput")
    bias_d = nc.dram_tensor("bias", [1, O_SH], dt.bfloat16, kind="ExternalInput")
    if nt_s:
        idx_d = nc.dram_tensor("cscidx", [128, nt_s, wpad], dt.int16,
                               kind="ExternalInput")
        val_d = nc.dram_tensor("cscval", [128, nt_s, wpad], dt.bfloat16,
                               kind="ExternalInput")
    if n_dense:
        atd_d = nc.dram_tensor("atd", [128, n_dense, O_SH], dt.bfloat16,
                               kind="ExternalInput")
    out_d = nc.dram_tensor("out", [NB, 128, O_SH], dt.float32,
                           kind="ExternalOutput")

    with tile.TileContext(nc) as tc:
        with (
            tc.tile_pool(name="const", bufs=1) as cpool,
            tc.tile_pool(name="at", bufs=6) as atpool,
            tc.tile_pool(name="psum", bufs=1, space="PSUM") as pspool,
            tc.tile_pool(name="outp", bufs=2) as opool,
        ):
            bias_sb = cpool.tile([1, O_SH], dt.bfloat16)
            nc.sync.dma_start(bias_sb[:], bias_d[:])
            ones_sb = cpool.tile([1, 128], dt.bfloat16)
            nc.vector.memset(ones_sb[:], 1.0)

            inT_sb = cpool.tile([128, NT, B], dt.bfloat16)
            # chunked so the first matmuls can start before all input lands
            CH = 4
            for c0 in range(0, NT, CH):
                nc.sync.dma_start(inT_sb[:, c0:c0 + CH, :],
                                  inT_d[:, c0:c0 + CH, :])

            if nt_s:
                idx_sb = cpool.tile([128, nt_s, wpad], dt.int16)
                val_sb = cpool.tile([128, nt_s, wpad], dt.bfloat16)
                nc.sync.dma_start(idx_sb[:], idx_d[:])
                nc.sync.dma_start(val_sb[:], val_d[:])

            # seed PSUM with bias broadcast across batch partitions:
            # psum = ones[1,128]^T @ bias[1,512]
            psums = []
            for i in range(NB):
                ps = pspool.tile([128, O_SH], dt.float32, tag=f"ps{i}")
                nc.tensor.matmul(
                    ps[:],
                    ones_sb[:],
                    bias_sb[:],
                    start=True, stop=False,
                )
                psums.append(ps)

            for t in range(NT):
                at = atpool.tile([128, O_SH], dt.bfloat16, tag="at")
                if t < n_dense:
                    nc.sync.dma_start(at[:], atd_d[:, t, :])
                else:
                    ts = t - n_dense
                    nc.gpsimd.local_scatter(
                        at[:],
                        val_sb[:, ts, :],
                        idx_sb[:, ts, :],
                        channels=128,
                        num_elems=O_SH,
                        num_idxs=wpad,
                    )
                last = t == NT - 1
                for i in range(NB):
                    nc.tensor.matmul(
                        psums[i][:],
                        inT_sb[:, t, 128 * i:128 * (i + 1)],
                        at[:],
                        start=False, stop=last,
                    )

            for i in range(NB):
                ot = opool.tile([128, O_SH], dt.float32, tag="out")
                nc.vector.tensor_copy(ot[:], psums[i][:])
                nc.sync.dma_start(out_d[i], ot[:])

    nc.compile()
    _prog_cache[key] = nc
    return nc


def _prepare(input, condensed_weight, input_mask, bias):
    """Host-side repack: dedupe + CSC-bin the sparse weights, cast/transpose
    the activations. Returns (in_maps, wpad, n_dense)."""
    # input^T bf16 tiled [128f, NT, B]: v[p, t, b] = input[b, 128t + p]
    inT = np.ascontiguousarray(
        input.astype(_BF16).T.reshape(NT, 128, B).transpose(1, 0, 2))

    # dedupe (o, f) pairs, summing weights in f64
    o_idx = np.repeat(np.arange(OUT_F, dtype=np.int64), K)
    f_idx = input_mask.ravel().astype(np.int64)
    w = condensed_weight.ravel()
    key = (o_idx << 12) | f_idx
    uk, inv = np.unique(key, return_inverse=True)
    sums = np.bincount(inv, weights=w.astype(np.float64))
    o_u = (uk >> 12).astype(np.int64)
    f_u = (uk & (IN_F - 1)).astype(np.int64)
    v_u = sums.astype(np.float32)

    core = o_u // O_SH
    o_loc = o_u % O_SH
    t_id = f_u // 128
    p_f = f_u % 128

    n_dense = N_DENSE_TILES
    nt_s = NT - n_dense

    dense_m = t_id < n_dense
    if n_dense:
        atd = np.zeros((N_CORES, 128, n_dense, O_SH), dtype=_BF16)
        atd[core[dense_m], p_f[dense_m], t_id[dense_m], o_loc[dense_m]] = \
            v_u[dense_m]

    wpad = 2
    if nt_s:
        sm = ~dense_m
        s_core, s_p, s_t, s_o, s_v = (core[sm], p_f[sm], t_id[sm] - n_dense,
                                      o_loc[sm], v_u[sm])
        # rank of each entry within its (core, feature) group
        g = ((s_core * 128 + s_p) * nt_s + s_t)
        order = np.argsort(g, kind="stable")
        gs = g[order]
        change = np.r_[True, gs[1:] != gs[:-1]]
        seg_start = np.flatnonzero(change)
        seg_id = np.cumsum(change) - 1
        rank = np.arange(gs.size) - seg_start[seg_id]

        maxc = int(rank.max()) + 1 if gs.size else 0
        wpad = max(2, (maxc + 1) // 2 * 2)

        idx_arr = np.full((N_CORES, 128, nt_s, wpad), -1, dtype=np.int16)
        val_arr = np.zeros((N_CORES, 128, nt_s, wpad), dtype=_BF16)
        idx_arr[s_core[order], s_p[order], s_t[order], rank] = \
            s_o[order].astype(np.int16)
        val_arr[s_core[order], s_p[order], s_t[order], rank] = s_v[order]

    in_maps = []
    for c in range(N_CORES):
        m = {
            "inT": inT,
            "bias": np.ascontiguousarray(
                bias[c * O_SH:(c + 1) * O_SH].reshape(1, O_SH)
            ).astype(_BF16),
        }
        if nt_s:
            m["cscidx"] = np.ascontiguousarray(idx_arr[c])
            m["cscval"] = np.ascontiguousarray(val_arr[c])
        if n_dense:
            m["atd"] = np.ascontiguousarray(atd[c])
        in_maps.append(m)
    return in_maps, wpad, n_dense


def kernel(input, condensed_weight, input_mask, bias,
           _run_kwargs=None, _res_box=None):
    """Full inputs in, full output out. Shards over 8 NeuronCores inside."""
    from concourse.bass_utils import run_bass_kernel_spmd

    in_maps, wpad, n_dense = _prepare(
        np.asarray(input), np.asarray(condensed_weight),
        np.asarray(input_mask), np.asarray(bias))
    nc = _build_program(wpad, n_dense)

    res = run_bass_kernel_spmd(nc, in_maps, list(range(N_CORES)),
                               **(_run_kwargs or {}))
    if _res_box is not None:
        _res_box["results"] = res

    out = np.concatenate(
        [np.asarray(res.results[c]["out"]).reshape(B, O_SH)
         for c in range(N_CORES)], axis=1)
    return out.astype(np.float32)


# revision 8
# speedup vs baseline: 1.2426x; 1.2426x over previous
"""CondensedLinearFineGrained on 8 TRN2 NeuronCores.

Math: out[b,o] = sum_k W[o,k] * input[b, mask[o,k]] + bias[o]
with B=256, IN_F=4096, OUT_F=4096, K=256.

Strategy
--------
Reformulate as a dense matmul:  out = input @ A^T + bias  where
A[o,f] = sum_{k: mask[o,k]==f} W[o,k]  (duplicates within a row are summed).

Materializing the gathered tensor [B, OUT_F, K] would push 268M elements
through the vector/gpsimd engines (hundreds of us); the dense reformulation
does 16x the MACs but on the 128x128 systolic array (~13.7us/core bf16),
and A is only 16x the size of W.

Sharding: output neurons, 512 per core. Per core:
  - input^T arrives as bf16 f-major tiles [128f x 32t x 256b] (2MB)
  - A^T f-tiles [128f x 512o] bf16 are built on-device by gpsimd
    local_scatter from host-repacked CSC (per-feature (o,weight) lists,
    deduped, -1-padded, int16 indices). Scattered f-tiles are built in
    PAIRS (one [128 x 1024] local_scatter per two tiles) to halve the
    Pool-engine instruction/semaphore overhead. The first N_DENSE_TILES
    f-tiles are instead shipped pre-densified from the host (bulk DMA
    into a resident buffer) to balance Pool time against spare DMA
    bandwidth.
  - TensorE accumulates psum[128b x 512o] over the 32 f-tiles; PSUM is
    seeded with bias via a K=1 bf16 matmul (ones^T @ bias broadcasts it
    across partitions).
"""

import numpy as np
import ml_dtypes

B = 256
IN_F = 4096
OUT_F = 4096
K = 256
N_CORES = 8
O_SH = OUT_F // N_CORES  # 512 output rows per core
NT = IN_F // 128         # 32 feature tiles
NB = B // 128            # 2 batch tiles

# f-tiles [0, N_DENSE_TILES) are DMA'd pre-densified; the rest are scattered
# on-device by gpsimd, two tiles per local_scatter. NT - N_DENSE_TILES must
# be even.
N_DENSE_TILES = 20
AT_BUFS = 6
IN_CH = 8  # inT DMA chunk, in f-tiles

_BF16 = ml_dtypes.bfloat16

_prog_cache = {}


def _build_program(wpad: int, n_dense: int):
    """Build + compile the SPMD Bass program. Cached on (wpad, n_dense)."""
    key = (wpad, n_dense)
    if key in _prog_cache:
        return _prog_cache[key]

    from concourse import bacc, tile, mybir

    nt_s = NT - n_dense      # scattered f-tiles
    npair = nt_s // 2        # local_scatter instructions
    assert nt_s % 2 == 0

    nc = bacc.Bacc("TRN2", target_bir_lowering=False, debug=False)
    dt = mybir.dt

    inT_d = nc.dram_tensor("inT", [128, NT, B], dt.bfloat16, kind="ExternalInput")
    bias_d = nc.dram_tensor("bias", [1, O_SH], dt.bfloat16, kind="ExternalInput")
    if npair:
        idx_d = nc.dram_tensor("cscidx", [128, npair, wpad], dt.int16,
                               kind="ExternalInput")
        val_d = nc.dram_tensor("cscval", [128, npair, wpad], dt.bfloat16,
                               kind="ExternalInput")
    if n_dense:
        atd_d = nc.dram_tensor("atd", [128, n_dense, O_SH], dt.bfloat16,
                               kind="ExternalInput")
    out_d = nc.dram_tensor("out", [NB, 128, O_SH], dt.float32,
                           kind="ExternalOutput")

    with tile.TileContext(nc) as tc:
        with (
            tc.tile_pool(name="const", bufs=1) as cpool,
            tc.tile_pool(name="at", bufs=AT_BUFS) as atpool,
            tc.tile_pool(name="psum", bufs=1, space="PSUM") as pspool,
            tc.tile_pool(name="outp", bufs=2) as opool,
        ):
            bias_sb = cpool.tile([1, O_SH], dt.bfloat16)
            nc.sync.dma_start(bias_sb[:], bias_d[:])
            ones_sb = cpool.tile([1, 128], dt.bfloat16)
            nc.vector.memset(ones_sb[:], 1.0)

            inT_sb = cpool.tile([128, NT, B], dt.bfloat16)
            # chunked so the first matmuls can start before all input lands
            for c0 in range(0, NT, IN_CH):
                nc.sync.dma_start(inT_sb[:, c0:c0 + IN_CH, :],
                                  inT_d[:, c0:c0 + IN_CH, :])

            if npair:
                idx_sb = cpool.tile([128, npair, wpad], dt.int16)
                val_sb = cpool.tile([128, npair, wpad], dt.bfloat16)
                nc.sync.dma_start(idx_sb[:], idx_d[:])
                nc.sync.dma_start(val_sb[:], val_d[:])

            if n_dense:
                atd_sb = cpool.tile([128, n_dense, O_SH], dt.bfloat16)
                h = n_dense // 2
                nc.sync.dma_start(atd_sb[:, :h, :], atd_d[:, :h, :])
                nc.sync.dma_start(atd_sb[:, h:, :], atd_d[:, h:, :])

            # seed PSUM with bias broadcast across batch partitions:
            # psum = ones[1,128]^T @ bias[1,512]
            psums = []
            for i in range(NB):
                ps = pspool.tile([128, O_SH], dt.float32, tag=f"ps{i}")
                nc.tensor.matmul(ps[:], ones_sb[:], bias_sb[:],
                                 start=True, stop=False)
                psums.append(ps)

            def mm(t, rhs_ap):
                last = t == NT - 1
                for i in range(NB):
                    nc.tensor.matmul(
                        psums[i][:],
                        inT_sb[:, t, 128 * i:128 * (i + 1)],
                        rhs_ap,
                        start=False, stop=last,
                    )

            for t in range(n_dense):
                mm(t, atd_sb[:, t, :])

            for j in range(npair):
                at = atpool.tile([128, 2, O_SH], dt.bfloat16, tag="at")
                nc.gpsimd.local_scatter(
                    at[:],
                    val_sb[:, j, :],
                    idx_sb[:, j, :],
                    channels=128,
                    num_elems=2 * O_SH,
                    num_idxs=wpad,
                )
                mm(n_dense + 2 * j, at[:, 0, :])
                mm(n_dense + 2 * j + 1, at[:, 1, :])

            for i in range(NB):
                ot = opool.tile([128, O_SH], dt.float32, tag="out")
                nc.vector.tensor_copy(ot[:], psums[i][:])
                nc.sync.dma_start(out_d[i], ot[:])

    nc.compile()
    _prog_cache[key] = nc
    return nc


def _prepare(input, condensed_weight, input_mask, bias):
    """Host-side repack: dedupe + CSC-bin the sparse weights, cast/transpose
    the activations. Returns (in_maps, wpad, n_dense)."""
    # input^T bf16 tiled [128f, NT, B]: v[p, t, b] = input[b, 128t + p]
    inT = np.ascontiguousarray(
        input.astype(_BF16).T.reshape(NT, 128, B).transpose(1, 0, 2))

    # dedupe (o, f) pairs, summing weights in f64
    o_idx = np.repeat(np.arange(OUT_F, dtype=np.int64), K)
    f_idx = input_mask.ravel().astype(np.int64)
    w = condensed_weight.ravel()
    key = (o_idx << 12) | f_idx
    uk, inv = np.unique(key, return_inverse=True)
    sums = np.bincount(inv, weights=w.astype(np.float64))
    o_u = (uk >> 12).astype(np.int64)
    f_u = (uk & (IN_F - 1)).astype(np.int64)
    v_u = sums.astype(np.float32)

    core = o_u // O_SH
    o_loc = o_u % O_SH
    t_id = f_u // 128
    p_f = f_u % 128

    n_dense = N_DENSE_TILES
    nt_s = NT - n_dense
    npair = nt_s // 2

    dense_m = t_id < n_dense
    if n_dense:
        atd = np.zeros((N_CORES, 128, n_dense, O_SH), dtype=_BF16)
        atd[core[dense_m], p_f[dense_m], t_id[dense_m], o_loc[dense_m]] = \
            v_u[dense_m]

    wpad = 2
    if npair:
        sm = ~dense_m
        ts = t_id[sm] - n_dense
        s_core, s_p, s_o, s_v = core[sm], p_f[sm], o_loc[sm], v_u[sm]
        s_pair = ts // 2
        # index within the merged pair tile: second tile offset by O_SH
        s_idx = s_o + O_SH * (ts % 2)
        # rank of each entry within its (core, partition, pair) group
        g = (s_core * 128 + s_p) * npair + s_pair
        order = np.argsort(g, kind="stable")
        gs = g[order]
        change = np.r_[True, gs[1:] != gs[:-1]]
        seg_start = np.flatnonzero(change)
        seg_id = np.cumsum(change) - 1
        rank = np.arange(gs.size) - seg_start[seg_id]

        maxc = int(rank.max()) + 1 if gs.size else 0
        wpad = max(2, (maxc + 1) // 2 * 2)

        idx_arr = np.full((N_CORES, 128, npair, wpad), -1, dtype=np.int16)
        val_arr = np.zeros((N_CORES, 128, npair, wpad), dtype=_BF16)
        idx_arr[s_core[order], s_p[order], s_pair[order], rank] = \
            s_idx[order].astype(np.int16)
        val_arr[s_core[order], s_p[order], s_pair[order], rank] = s_v[order]

    in_maps = []
    for c in range(N_CORES):
        m = {
            "inT": inT,
            "bias": np.ascontiguousarray(
                bias[c * O_SH:(c + 1) * O_SH].reshape(1, O_SH)
            ).astype(_BF16),
        }
        if npair:
            m["cscidx"] = np.ascontiguousarray(idx_arr[c])
            m["cscval"] = np.ascontiguousarray(val_arr[c])
        if n_dense:
            m["atd"] = np.ascontiguousarray(atd[c])
        in_maps.append(m)
    return in_maps, wpad, n_dense


def kernel(input, condensed_weight, input_mask, bias,
           _run_kwargs=None, _res_box=None):
    """Full inputs in, full output out. Shards over 8 NeuronCores inside."""
    from concourse.bass_utils import run_bass_kernel_spmd

    in_maps, wpad, n_dense = _prepare(
        np.asarray(input), np.asarray(condensed_weight),
        np.asarray(input_mask), np.asarray(bias))
    nc = _build_program(wpad, n_dense)

    res = run_bass_kernel_spmd(nc, in_maps, list(range(N_CORES)),
                               **(_run_kwargs or {}))
    if _res_box is not None:
        _res_box["results"] = res

    out = np.concatenate(
        [np.asarray(res.results[c]["out"]).reshape(B, O_SH)
         for c in range(N_CORES)], axis=1)
    return out.astype(np.float32)


# revision 11
# speedup vs baseline: 1.5180x; 1.2216x over previous
"""CondensedLinearFineGrained on 8 TRN2 NeuronCores.

Math: out[b,o] = sum_k W[o,k] * input[b, mask[o,k]] + bias[o]
with B=256, IN_F=4096, OUT_F=4096, K=256.

Strategy
--------
Reformulate as a dense matmul:  out = input @ A^T + bias  where
A[o,f] = sum_{k: mask[o,k]==f} W[o,k]  (duplicates within a row are summed).

Materializing the gathered tensor [B, OUT_F, K] would push 268M elements
through the vector/gpsimd engines (hundreds of us); the dense reformulation
does 16x the MACs but on the 128x128 systolic array (~13.7us/core bf16),
and A is only 16x the size of W.

Sharding: output neurons, 512 per core. Per core:
  - input^T arrives as bf16 f-major tiles [128f x 32t x 256b] (2MB)
  - A^T f-tiles [128f x 512o] bf16 are built on-device by gpsimd
    local_scatter from host-repacked CSC (per-feature (o,weight) lists,
    deduped, -1-padded, int16 indices). Scattered f-tiles are built in
    PAIRS (one [128 x 1024] local_scatter per two tiles) to halve the
    Pool-engine instruction/semaphore overhead. The first N_DENSE_TILES
    f-tiles are instead shipped pre-densified from the host (bulk DMA
    into a resident buffer) to balance Pool time against spare DMA
    bandwidth.
  - TensorE accumulates psum[128b x 512o] over the 32 f-tiles; PSUM is
    seeded with bias via a K=1 bf16 matmul (ones^T @ bias broadcasts it
    across partitions).
"""

import numpy as np
import ml_dtypes

B = 256
IN_F = 4096
OUT_F = 4096
K = 256
N_CORES = 8
O_SH = OUT_F // N_CORES  # 512 output rows per core
NT = IN_F // 128         # 32 feature tiles
NB = B // 128            # 2 batch tiles

# f-tiles [0, N_DENSE_TILES) are DMA'd pre-densified; the rest are scattered
# on-device by gpsimd, two tiles per local_scatter. NT - N_DENSE_TILES must
# be even.
N_DENSE_TILES = 22
IN_CH = 8   # inT DMA chunk, in f-tiles
ATD_CH = 4  # atd DMA chunk, in f-tiles

_BF16 = ml_dtypes.bfloat16

_prog_cache = {}


def _build_program(wpad: int, n_dense: int):
    """Build + compile the SPMD Bass program. Cached on (wpad, n_dense)."""
    key = (wpad, n_dense)
    if key in _prog_cache:
        return _prog_cache[key]

    from concourse import bacc, tile, mybir

    nt_s = NT - n_dense      # scattered f-tiles
    npair = nt_s // 2        # local_scatter instructions
    assert nt_s % 2 == 0

    nc = bacc.Bacc("TRN2", target_bir_lowering=False, debug=False)
    dt = mybir.dt

    inT_d = nc.dram_tensor("inT", [128, NT, B], dt.bfloat16, kind="ExternalInput")
    bias_d = nc.dram_tensor("bias", [1, O_SH], dt.bfloat16, kind="ExternalInput")
    if npair:
        idx_d = nc.dram_tensor("cscidx", [128, npair, wpad], dt.int16,
                               kind="ExternalInput")
        val_d = nc.dram_tensor("cscval", [128, npair, wpad], dt.bfloat16,
                               kind="ExternalInput")
    if n_dense:
        atd_d = nc.dram_tensor("atd", [128, n_dense, O_SH], dt.bfloat16,
                               kind="ExternalInput")
    out_d = nc.dram_tensor("out", [NB, 128, O_SH], dt.float32,
                           kind="ExternalOutput")

    with tile.TileContext(nc) as tc:
        with (
            tc.tile_pool(name="const", bufs=1) as cpool,
            tc.tile_pool(name="at", bufs=max(npair, 1)) as atpool,
            tc.tile_pool(name="psum", bufs=1, space="PSUM") as pspool,
            tc.tile_pool(name="outp", bufs=2) as opool,
        ):
            # DMA issue is spread over the two HWDGE engines so their
            # queues run concurrently: sync = CSC (small, unblocks the
            # gpsimd scatters immediately) + dense A^T stream (first
            # consumed by PE), scalar = bias + input stream.
            if npair:
                idx_sb = cpool.tile([128, npair, wpad], dt.int16)
                val_sb = cpool.tile([128, npair, wpad], dt.bfloat16)
                nc.sync.dma_start(idx_sb[:], idx_d[:])
                nc.sync.dma_start(val_sb[:], val_d[:])

            bias_sb = cpool.tile([1, O_SH], dt.bfloat16)
            nc.scalar.dma_start(bias_sb[:], bias_d[:])
            ones_sb = cpool.tile([1, 128], dt.bfloat16)
            nc.vector.memset(ones_sb[:], 1.0)

            inT_sb = cpool.tile([128, NT, B], dt.bfloat16)
            for c0 in range(0, NT, IN_CH):
                nc.scalar.dma_start(inT_sb[:, c0:c0 + IN_CH, :],
                                    inT_d[:, c0:c0 + IN_CH, :])

            if n_dense:
                atd_sb = cpool.tile([128, n_dense, O_SH], dt.bfloat16)
                for c0 in range(0, n_dense, ATD_CH):
                    c1 = min(c0 + ATD_CH, n_dense)
                    nc.sync.dma_start(atd_sb[:, c0:c1, :], atd_d[:, c0:c1, :])

            # seed PSUM with bias broadcast across batch partitions:
            # psum = ones[1,128]^T @ bias[1,512]
            psums = []
            for i in range(NB):
                ps = pspool.tile([128, O_SH], dt.float32, tag=f"ps{i}")
                nc.tensor.matmul(ps[:], ones_sb[:], bias_sb[:],
                                 start=True, stop=False)
                psums.append(ps)

            def mm(t, rhs_ap):
                last = t == NT - 1
                for i in range(NB):
                    nc.tensor.matmul(
                        psums[i][:],
                        inT_sb[:, t, 128 * i:128 * (i + 1)],
                        rhs_ap,
                        start=False, stop=last,
                    )

            for t in range(n_dense):
                mm(t, atd_sb[:, t, :])

            for j in range(npair):
                at = atpool.tile([128, 2, O_SH], dt.bfloat16, tag="at")
                nc.gpsimd.local_scatter(
                    at[:],
                    val_sb[:, j, :],
                    idx_sb[:, j, :],
                    channels=128,
                    num_elems=2 * O_SH,
                    num_idxs=wpad,
                )
                mm(n_dense + 2 * j, at[:, 0, :])
                mm(n_dense + 2 * j + 1, at[:, 1, :])

            for i in range(NB):
                ot = opool.tile([128, O_SH], dt.float32, tag="out")
                nc.vector.tensor_copy(ot[:], psums[i][:])
                nc.sync.dma_start(out_d[i], ot[:])

    nc.compile()
    _prog_cache[key] = nc
    return nc


def _prepare(input, condensed_weight, input_mask, bias):
    """Host-side repack: dedupe + CSC-bin the sparse weights, cast/transpose
    the activations. Returns (in_maps, wpad, n_dense)."""
    # input^T bf16 tiled [128f, NT, B]: v[p, t, b] = input[b, 128t + p]
    inT = np.ascontiguousarray(
        input.astype(_BF16).T.reshape(NT, 128, B).transpose(1, 0, 2))

    # dedupe (o, f) pairs, summing weights in f64
    o_idx = np.repeat(np.arange(OUT_F, dtype=np.int64), K)
    f_idx = input_mask.ravel().astype(np.int64)
    w = condensed_weight.ravel()
    key = (o_idx << 12) | f_idx
    uk, inv = np.unique(key, return_inverse=True)
    sums = np.bincount(inv, weights=w.astype(np.float64))
    o_u = (uk >> 12).astype(np.int64)
    f_u = (uk & (IN_F - 1)).astype(np.int64)
    v_u = sums.astype(np.float32)

    core = o_u // O_SH
    o_loc = o_u % O_SH
    t_id = f_u // 128
    p_f = f_u % 128

    n_dense = N_DENSE_TILES
    nt_s = NT - n_dense
    npair = nt_s // 2

    dense_m = t_id < n_dense
    if n_dense:
        atd = np.zeros((N_CORES, 128, n_dense, O_SH), dtype=_BF16)
        atd[core[dense_m], p_f[dense_m], t_id[dense_m], o_loc[dense_m]] = \
            v_u[dense_m]

    wpad = 2
    if npair:
        sm = ~dense_m
        ts = t_id[sm] - n_dense
        s_core, s_p, s_o, s_v = core[sm], p_f[sm], o_loc[sm], v_u[sm]
        s_pair = ts // 2
        # index within the merged pair tile: second tile offset by O_SH
        s_idx = s_o + O_SH * (ts % 2)
        # rank of each entry within its (core, partition, pair) group
        g = (s_core * 128 + s_p) * npair + s_pair
        order = np.argsort(g, kind="stable")
        gs = g[order]
        change = np.r_[True, gs[1:] != gs[:-1]]
        seg_start = np.flatnonzero(change)
        seg_id = np.cumsum(change) - 1
        rank = np.arange(gs.size) - seg_start[seg_id]

        maxc = int(rank.max()) + 1 if gs.size else 0
        wpad = max(2, (maxc + 1) // 2 * 2)

        idx_arr = np.full((N_CORES, 128, npair, wpad), -1, dtype=np.int16)
        val_arr = np.zeros((N_CORES, 128, npair, wpad), dtype=_BF16)
        idx_arr[s_core[order], s_p[order], s_pair[order], rank] = \
            s_idx[order].astype(np.int16)
        val_arr[s_core[order], s_p[order], s_pair[order], rank] = s_v[order]

    in_maps = []
    for c in range(N_CORES):
        m = {
            "inT": inT,
            "bias": np.ascontiguousarray(
                bias[c * O_SH:(c + 1) * O_SH].reshape(1, O_SH)
            ).astype(_BF16),
        }
        if npair:
            m["cscidx"] = np.ascontiguousarray(idx_arr[c])
            m["cscval"] = np.ascontiguousarray(val_arr[c])
        if n_dense:
            m["atd"] = np.ascontiguousarray(atd[c])
        in_maps.append(m)
    return in_maps, wpad, n_dense


def kernel(input, condensed_weight, input_mask, bias,
           _run_kwargs=None, _res_box=None):
    """Full inputs in, full output out. Shards over 8 NeuronCores inside."""
    from concourse.bass_utils import run_bass_kernel_spmd

    in_maps, wpad, n_dense = _prepare(
        np.asarray(input), np.asarray(condensed_weight),
        np.asarray(input_mask), np.asarray(bias))
    nc = _build_program(wpad, n_dense)

    res = run_bass_kernel_spmd(nc, in_maps, list(range(N_CORES)),
                               **(_run_kwargs or {}))
    if _res_box is not None:
        _res_box["results"] = res

    out = np.concatenate(
        [np.asarray(res.results[c]["out"]).reshape(B, O_SH)
         for c in range(N_CORES)], axis=1)
    return out.astype(np.float32)
